# revision 14
# baseline (speedup 1.0000x reference)
"""Trainium2 Bass kernel for AttentionMemorySystem (circular-buffer scatter).

Reference semantics (B=16384, D=256, M=262144):
    idx        = (memory_index + arange(B)) % M          # contiguous window
    new_mem    = memory_attentions.at[idx].set(features)
    new_util   = memory_utilities.at[idx].set(attention_quality)
    utilization, quality (scalars)

Sharding: rotated cyclic rows across 8 cores. Core c owns global rows r with
(r - memory_index) % 8 == c, ordered by virtual row v = (r - memory_index) % M,
local l = v // 8.  Under this layout every core's write window is local rows
[0, B/8) — a single static SPMD program; each core copies ~32MB in + ~32MB out.

Raw Bass (no Tile): TRN2 codegen allows at most one embedded sync-wait per
instruction, so all bulk DMAs share one semaphore, the quality chain is
serialised on a second, and the tail barrier is a single wait per engine.
"""

import numpy as np

B, D, M, NCORES = 16384, 256, 262144, 8
RPC = M // NCORES  # 32768 rows of memory per core
FPC = B // NCORES  # 2048 feature rows per core
KREP = 128         # host-provided aq pattern width; device replicates x16

LAST_RESULTS = None  # BassKernelResults of the most recent run (for test harness)

_cache = {}


def _build(tail_rows: int, repeat: int = 1):
    """Build the SPMD Bass program. tail_rows>0 additionally computes the
    per-core partial sum for `quality` (tail of local util shard = old rows).
    repeat>1 unrolls the bulk-DMA block (idempotent) for benchmarking."""
    import concourse.bass as bass
    import concourse.mybir as mybir
    from contextlib import ExitStack

    f32 = mybir.dt.float32
    nc = bass.Bass()

    # local rows [0:FPC) of mem_out come from feat_in; [FPC:RPC) from mem_in
    mem_in = nc.dram_tensor("mem_in", [RPC - FPC, D], f32, kind="ExternalInput")
    feat_in = nc.dram_tensor("feat_in", [FPC, D], f32, kind="ExternalInput")
    util_in = nc.dram_tensor("util_in", [1, RPC - FPC], f32, kind="ExternalInput")
    # aqrep = attention_quality replicated KREP times (device replicates x16)
    aqrep = nc.dram_tensor("aqrep", [1, KREP], f32, kind="ExternalInput")

    mem_out = nc.dram_tensor("mem_out", [RPC, D], f32, kind="ExternalOutput")
    util_out = nc.dram_tensor("util_out", [1, RPC], f32, kind="ExternalOutput")

    if tail_rows > 0:
        # qin = [tail utils (old rows), aq pattern (KREP)] in one tensor so the
        # whole quality computation hangs off a single DMA.
        qin = nc.dram_tensor("qin", [1, tail_rows + KREP], f32, kind="ExternalInput")
        qsum = nc.dram_tensor("qsum", [1, 1], f32, kind="ExternalOutput")

    with ExitStack() as ctx:
        dma_sem = ctx.enter_context(nc.semaphore("dma_sem"))
        adma_sem = ctx.enter_context(nc.semaphore("adma_sem"))
        if tail_rows > 0:
            qin_sem = ctx.enter_context(nc.semaphore("qin_sem"))
            dve_sem = ctx.enter_context(nc.semaphore("dve_sem"))
            t_q = ctx.enter_context(
                nc.sbuf_tensor("t_q", [1, tail_rows + KREP], f32)
            )
            r1 = ctx.enter_context(nc.sbuf_tensor("r1", [1, 1], f32))
            r2 = ctx.enter_context(nc.sbuf_tensor("r2", [1, 1], f32))
            r2x = ctx.enter_context(nc.sbuf_tensor("r2x", [1, 1], f32))
            t_o = ctx.enter_context(nc.sbuf_tensor("t_o", [1, 1], f32))
        block = ctx.enter_context(nc.Block())

        # NB: raw HWDGE (sync/scalar) dma_start hangs under this runtime —
        # use SWDGE (gpsimd) for every DMA (cf. test_sync_dma_collective_hang).
        @block.gpsimd
        def _(gpsimd):
            if tail_rows > 0:
                gpsimd.dma_start(out=t_q[:, :], in_=qin[:, :]).then_inc(qin_sem, 16)
            dma_incs = 0
            for _r in range(repeat):
                # bulk copy: local rows [FPC:RPC) of memory + utilities
                gpsimd.dma_start(out=mem_out[0:FPC, :], in_=feat_in[:, :]).then_inc(
                    adma_sem, 16
                )
                gpsimd.dma_start(out=mem_out[FPC:RPC, :], in_=mem_in[:, :]).then_inc(
                    dma_sem, 16
                )
                gpsimd.dma_start(
                    out=util_out[:, FPC:RPC], in_=util_in[:, :]
                ).then_inc(dma_sem, 16)
                # fill util_out[0:FPC] with aq: replicate the KREP pattern x16
                rep = bass.AP(aqrep, 0, [[0, FPC // KREP], [1, KREP]])
                gpsimd.dma_start(out=util_out[:, 0:FPC], in_=rep).then_inc(
                    dma_sem, 16
                )
                dma_incs += 48
            if tail_rows > 0:
                gpsimd.wait_ge(dve_sem, 4)
                gpsimd.dma_start(out=qsum[:, :], in_=t_o[:, :]).then_inc(dma_sem, 16)
                dma_incs += 16
            gpsimd.wait_ge(dma_sem, dma_incs)
            gpsimd.wait_ge(adma_sem, 16 * repeat)

        if tail_rows > 0:

            @block.vector
            def _(vector):
                vector.wait_ge(qin_sem, 16)
                nc.vector.reduce_sum(
                    r1[:, :], t_q[:, 0:tail_rows], axis=mybir.AxisListType.X
                ).then_inc(dve_sem)
                vector.wait_ge(dve_sem, 1)
                nc.vector.reduce_sum(
                    r2[:, :],
                    t_q[:, tail_rows : tail_rows + KREP],
                    axis=mybir.AxisListType.X,
                ).then_inc(dve_sem)
                vector.wait_ge(dve_sem, 2)
                nc.vector.tensor_scalar_mul(
                    r2x[:, :], r2[:, :], float(FPC // KREP)
                ).then_inc(dve_sem)
                vector.wait_ge(dve_sem, 3)
                nc.vector.tensor_add(t_o[:, :], r1[:, :], r2x[:, :]).then_inc(
                    dve_sem
                )

    return nc


def _get_nc(tail_rows: int, repeat: int = 1):
    key = (tail_rows, repeat)
    if key not in _cache:
        _cache[key] = _build(tail_rows, repeat)
    return _cache[key]


def kernel(features, attention_quality, memory_attentions, memory_utilities, memory_index):
    global LAST_RESULTS
    from concourse.bass_utils import run_bass_kernel_spmd

    features = np.asarray(features, dtype=np.float32)
    aq = np.float32(np.asarray(attention_quality).reshape(()))
    memory_attentions = np.asarray(memory_attentions, dtype=np.float32)
    memory_utilities = np.asarray(memory_utilities, dtype=np.float32)
    idx = int(memory_index)

    assert features.shape == (B, D) and memory_attentions.shape == (M, D)
    assert B % NCORES == 0 and M % NCORES == 0

    new_index = (idx + B) % M
    full = (idx + B) >= M
    # device quality path only for the aligned, non-wrapping case
    device_quality = (idx % NCORES == 0) and not full
    tail_rows = (idx // NCORES) if device_quality else 0

    nc = _get_nc(tail_rows)
    aqrep = np.full((1, KREP), aq, dtype=np.float32)

    # --- shard (host) ---
    # core c, virtual rows v = c, c+8, ... ; global r = (idx + v) % M.
    # The walk splits into two plain strided slices of the global array.
    in_maps = []
    seg_meta = []
    for c in range(NCORES):
        start = (idx + c) % M
        n1 = len(range(start, M, NCORES))
        # local rows [FPC:RPC) = virtual rows [B:M) for this core; global row
        # of local l is (idx + 8l + c) % M. Only these are read by the device.
        mem_shard = np.empty((RPC, D), dtype=np.float32)
        mem_shard[:n1] = memory_attentions[start::NCORES]
        mem_shard[n1:] = memory_attentions[start % NCORES : start : NCORES]
        util_shard = np.empty((1, RPC), dtype=np.float32)
        util_shard[0, :n1] = memory_utilities[start::NCORES]
        util_shard[0, n1:] = memory_utilities[start % NCORES : start : NCORES]
        im = {
            "mem_in": mem_shard[FPC:],
            "feat_in": np.ascontiguousarray(features[c::NCORES]),
            "util_in": util_shard[:, FPC:],
            "aqrep": aqrep,
        }
        if tail_rows > 0:
            im["qin"] = np.concatenate(
                [util_shard[:, RPC - tail_rows :], aqrep], axis=1
            )
        in_maps.append(im)
        seg_meta.append((start, n1))

    # --- run on 8 NeuronCores ---
    res = run_bass_kernel_spmd(nc, in_maps, core_ids=list(range(NCORES)))
    LAST_RESULTS = res

    # --- gather (host) ---
    new_mem = np.empty((M, D), dtype=np.float32)
    new_util = np.empty((M,), dtype=np.float32)
    qsum_total = 0.0
    for c in range(NCORES):
        start, n1 = seg_meta[c]
        r = res.results[c]
        mo = r["mem_out"]
        uo = r["util_out"].reshape(-1)
        new_mem[start::NCORES] = mo[:n1]
        new_mem[start % NCORES : start : NCORES] = mo[n1:]
        new_util[start::NCORES] = uo[:n1]
        new_util[start % NCORES : start : NCORES] = uo[n1:]
        if tail_rows > 0:
            qsum_total += float(r["qsum"][0, 0])

    utilization = np.float32(1.0 if full else new_index / M)
    if device_quality:
        quality = np.float32(qsum_total / new_index)
    else:
        quality = np.float32(np.mean(new_util[:new_index], dtype=np.float32))

    return new_mem, new_util, utilization, quality
